# revision 9
# baseline (speedup 1.0000x reference)
"""Biased attention Trainium2 kernel, SPMD over 8 NeuronCores.

Problem (per reference):
    sim  = q @ k^T / sqrt(64)                       [b,h,i,j]
    sim  = where(mask[b,j], sim, -fmax)
    sim -= taus[h] * attn_bias[b,i,j]
    out  = softmax(sim, axis=j) @ v                 [b,h,i,d]

Shapes: B=2, H=16, S=2048, D=64, fp32.

Sharding: batch*heads across 8 cores -> 4 (b,h) pairs per core, all with
the same batch b (core c handles b=c//4, heads 4*(c%4)..4*(c%4)+3), so
attn_bias/mask are batch-sharded and loaded once per core.

Per-core dataflow (all on device):
  - scores are computed TRANSPOSED: zT[j,i] = K Q^T, j on partitions, so
    softmax reductions run along the matmul contraction instead of needing
    a big transpose of the attention matrix.
  - key-padding mask folds into the softmax exp as a per-partition bias
    (maskadd[j] = (mask-1)*1e30) on the ACT activation instruction.
  - the tau*attn_bias subtraction is done two ways, load-balanced between
    engines: PE path (scaled-identity matmul accumulating -8*tau*biasT
    into the scores PSUM) and DVE path (scalar_tensor_tensor fused
    (biasT * -8tau) + scores). 1/sqrt(d)=1/8 folds into the exp scale.
  - V gets a ones-column appended so the softmax denominator falls out of
    the attention @ V matmul for free (row 64 of the [65,512] output).
  - output is un-transposed per 128-column block with PE transpose into
    the freed accumulator bank, then normalized with a per-partition
    reciprocal multiply and DMA'd out.
"""

import numpy as np
from contextlib import ExitStack

import concourse.bass as bass
import concourse.tile as tile
from concourse import bacc, mybir
from concourse import bass_utils

F32 = mybir.dt.float32
F32R = mybir.dt.float32r
BF16 = mybir.dt.bfloat16
U8 = mybir.dt.uint8
Alu = mybir.AluOpType
Act = mybir.ActivationFunctionType

B, H, S, D = 2, 16, 2048, 64
N_CORES = 8
HPC = 4          # heads per core
JT = S // 128    # 16 j-tiles of 128
NP = S // 512    # 4 i-panels of 512
BIG = 1.0e30

# dtype used for matmul operands (PE fast fp32 path); falls back to F32 if
# hardware/toolchain rejects it -- see _build().
MM_DT = F32R
# exp output dtype (feeds the attention@V matmul as the moving operand)
EXP_DT = F32R
# which heads use the DVE scalar_tensor_tensor bias path per (panel, jtile):
# STT on pair0 every j balances PE/DVE/ACT at roughly 2.1us per (P,j) group.
def stt_pairs(j):
    return {0}          # pair 0 -> DVE path; pair 1 -> PE identity path


def _build(n_rep=1):
    nc = bacc.Bacc("TRN2", target_bir_lowering=False, debug=False,
                   num_devices=N_CORES)

    q_ap = nc.dram_tensor("q", [HPC, S, D], F32, kind="ExternalInput").ap()
    k_ap = nc.dram_tensor("k", [HPC, S, D], F32, kind="ExternalInput").ap()
    v_ap = nc.dram_tensor("v", [HPC, S, D], F32, kind="ExternalInput").ap()
    taus_ap = nc.dram_tensor("taus", [HPC], F32, kind="ExternalInput").ap()
    mask_ap = nc.dram_tensor("mask", [S], U8, kind="ExternalInput").ap()
    bias_ap = nc.dram_tensor("bias", [S, S], F32, kind="ExternalInput").ap()
    out_ap = nc.dram_tensor("out", [HPC, S, D], F32, kind="ExternalOutput").ap()

    with tile.TileContext(nc) as tc:
        for _rep in range(n_rep):
            with ExitStack() as ctx:
                _body(ctx, tc, q_ap, k_ap, v_ap, taus_ap, mask_ap, bias_ap,
                      out_ap)

    nc.compile()
    return nc


def _body(ctx, tc, q_ap, k_ap, v_ap, taus_ap, mask_ap, bias_ap, out_ap):
    nc = tc.nc

    const = ctx.enter_context(tc.tile_pool(name="const", bufs=1))
    tmp = ctx.enter_context(tc.tile_pool(name="tmp", bufs=2))
    braw = ctx.enter_context(tc.tile_pool(name="braw", bufs=18))
    benc = ctx.enter_context(tc.tile_pool(name="benc", bufs=3))
    zsb = ctx.enter_context(tc.tile_pool(name="zsb", bufs=3))
    epool = ctx.enter_context(tc.tile_pool(name="epool", bufs=3))
    dpool = ctx.enter_context(tc.tile_pool(name="dpool", bufs=2))
    zps = ctx.enter_context(tc.tile_pool(name="zps", bufs=2, space="PSUM"))
    ops = ctx.enter_context(tc.tile_pool(name="ops", bufs=1, space="PSUM"))

    # ---- constants / per-head prep -------------------------------------
    # taus broadcast to all partitions, scaled by -8: one K=1 matmul.
    taus_s = const.tile([1, HPC], F32, tag="taus_s")
    nc.sync.dma_start(taus_s[:], taus_ap[None, :])
    ones_n8 = const.tile([1, 128], F32, tag="ones_n8")
    nc.vector.memset(ones_n8[:], -8.0)
    zp0 = zps.tile([128, 1024], F32, tag="zp")
    nc.tensor.matmul(zp0[:, 0:HPC], lhsT=ones_n8[:], rhs=taus_s[:],
                     start=True, stop=True)
    n8tau = const.tile([128, HPC], F32, tag="n8tau")
    nc.vector.tensor_copy(n8tau[:], zp0[:, 0:HPC])

    # identity (for PE transpose) and per-head scaled identities (-8*tau*I)
    ones_t = const.tile([128, 128], F32, tag="ones_t")
    nc.vector.memset(ones_t[:], 1.0)
    ident = const.tile([128, 128], F32, tag="ident")
    nc.gpsimd.affine_select(ident[:], ones_t[:], pattern=[[1, 128]], base=0,
                            channel_multiplier=-1, compare_op=Alu.is_equal,
                            fill=0.0)
    scaledI = const.tile([128, 128 * HPC], MM_DT, tag="scaledI")
    for h in range(HPC):
        nc.vector.tensor_scalar_mul(scaledI[:, h * 128:(h + 1) * 128],
                                    ident[:], n8tau[:, h:h + 1])

    # maskadd[j] = (mask - 1) * BIG as [128, JT] fp32
    m_u8 = const.tile([128, JT], U8, tag="m_u8")
    nc.sync.dma_start(m_u8[:], mask_ap.rearrange("(t p) -> p t", p=128))
    m_f = const.tile([128, JT], F32, tag="m_f")
    nc.vector.tensor_copy(m_f[:], m_u8[:])
    maskadd = const.tile([128, JT], F32, tag="maskadd")
    nc.vector.tensor_scalar(maskadd[:], m_f[:], 1.0, BIG,
                            op0=Alu.subtract, op1=Alu.mult)

    # Q^T / K^T head-pair tiles [128, S]: even head on partitions 0-63,
    # odd head on 64-127 (enables row-packed concurrent QK matmuls).
    qtr = []
    ktr = []
    for pair in range(2):
        qraw = tmp.tile([128, S], F32, tag="qkraw")
        nc.sync.dma_start(qraw[0:64, :], q_ap[2 * pair].rearrange("s d -> d s"))
        nc.sync.dma_start(qraw[64:128, :], q_ap[2 * pair + 1].rearrange("s d -> d s"))
        qt = const.tile([128, S], MM_DT, tag=f"qtr{pair}")
        nc.vector.tensor_copy(qt[:], qraw[:])
        qtr.append(qt)

        kraw = tmp.tile([128, S], F32, tag="qkraw")
        nc.sync.dma_start(kraw[0:64, :], k_ap[2 * pair].rearrange("s d -> d s"))
        nc.sync.dma_start(kraw[64:128, :], k_ap[2 * pair + 1].rearrange("s d -> d s"))
        kt = const.tile([128, S], MM_DT, tag=f"ktr{pair}")
        nc.vector.tensor_copy(kt[:], kraw[:])
        ktr.append(kt)

    # V with ones column: [128, JT*65] per head, col t*65+64 == 1.0
    vaug = []
    for h in range(HPC):
        vraw = tmp.tile([128, JT * 65], F32, tag="vraw")
        nc.vector.memset(vraw[:], 1.0)
        dst = vraw[:].rearrange("p (t c) -> p t c", c=65)[:, :, 0:64]
        nc.sync.dma_start(dst, v_ap[h].rearrange("(t p) d -> p t d", p=128))
        va = const.tile([128, JT * 65], MM_DT, tag=f"vaug{h}")
        nc.vector.tensor_copy(va[:], vraw[:])
        vaug.append(va)

    # ---- main loops ----------------------------------------------------
    # Head pairs are processed sequentially per panel (phase 0: heads 0/1,
    # phase 1: heads 2/3) so only 2 PSUM o-banks are live per phase and the
    # scores pool gets 2 [128,1024] slots for pipelining. Bias tiles are
    # cached raw in SBUF per panel and shared by both phases; the tau*bias
    # application alternates DVE (scalar_tensor_tensor) / PE (scaled
    # identity matmul) by j-parity to balance engine load.
    for P in range(NP):
        isl = slice(P * 512, (P + 1) * 512)
        bcache = [None] * JT
        for j in range(JT):
            jsl = slice(j * 128, (j + 1) * 128)
            bT = braw.tile([128, 512], F32, tag="bT", name=f"bT_{P}_{j}")
            nc.sync.dma_start(bT[:], bias_ap.rearrange("i j -> j i")[jsl, isl])
            bcache[j] = bT

        for pair in range(2):
            o = [ops.tile([128, 512], F32, tag=f"o{h}", name=f"o{h}_{P}")
                 for h in (2 * pair, 2 * pair + 1)]
            for j in range(JT):
                jsl = slice(j * 128, (j + 1) * 128)
                bT = bcache[j]
                use_stt = (j % 2 == 0)
                zp = zps.tile([128, 1024], F32, tag="zp", name=f"zp_{P}_{pair}_{j}")
                if not use_stt:
                    bTr = benc.tile([128, 512], MM_DT, tag="bTr", name=f"bTr_{P}_{pair}_{j}")
                    nc.vector.tensor_copy(bTr[:], bT[:])
                for t in range(2):
                    h = 2 * pair + t
                    psl = slice(t * 64, (t + 1) * 64)
                    zsl = slice(t * 512, (t + 1) * 512)
                    nc.tensor.matmul(zp[:, zsl], lhsT=ktr[pair][psl, jsl],
                                     rhs=qtr[pair][psl, isl],
                                     start=True, stop=use_stt)
                    if not use_stt:
                        nc.tensor.matmul(
                            zp[:, zsl],
                            lhsT=scaledI[:, h * 128:(h + 1) * 128],
                            rhs=bTr[:], start=False, stop=True,
                            skip_group_check=True)
                et = epool.tile([128, 1024], EXP_DT, tag="et", name=f"et_{P}_{pair}_{j}")
                if use_stt:
                    zs = zsb.tile([128, 1024], F32, tag="zs", name=f"zs_{P}_{pair}_{j}")
                    for t in range(2):
                        h = 2 * pair + t
                        zsl = slice(t * 512, (t + 1) * 512)
                        nc.vector.scalar_tensor_tensor(
                            zs[:, zsl], in0=bT[:], scalar=n8tau[:, h:h + 1],
                            in1=zp[:, zsl], op0=Alu.mult, op1=Alu.add)
                    nc.scalar.activation(et[:], zs[:], Act.Exp,
                                         bias=maskadd[:, j:j + 1], scale=0.125)
                else:
                    nc.scalar.activation(et[:], zp[:], Act.Exp,
                                         bias=maskadd[:, j:j + 1], scale=0.125)
                for t in range(2):
                    h = 2 * pair + t
                    nc.tensor.matmul(
                        o[t][0:65, :],
                        lhsT=vaug[h][:, j * 65:(j + 1) * 65],
                        rhs=et[:, t * 512:(t + 1) * 512],
                        start=(j == 0), stop=(j == JT - 1))

            # ---- drain: transpose + normalize + store ------------------
            for t in range(2):
                h = 2 * pair + t
                ob = dpool.tile([65, 512], F32, tag="ob", name=f"ob_{P}_{h}")
                nc.vector.tensor_copy(ob[:], o[t][0:65, :])
                for c in range(4):
                    nc.tensor.transpose(o[t][:, c * 65:(c + 1) * 65],
                                        ob[:, c * 128:(c + 1) * 128],
                                        ident[0:65, 0:65])
                oc = o[t][:, 0:260].rearrange("p (c x) -> p c x", x=65)
                rec = dpool.tile([128, 4], F32, tag="rec", name=f"rec_{P}_{h}")
                nc.vector.reciprocal(rec[:], oc[:, :, 64])
                ostage = dpool.tile([128, 256], F32, tag="ostage",
                                    name=f"ostage_{P}_{h}")
                nc.vector.tensor_tensor(
                    ostage[:].rearrange("p (c x) -> p c x", x=64),
                    oc[:, :, 0:64],
                    rec[:].broadcast_to((128, 4, 64)),
                    op=Alu.mult)
                nc.sync.dma_start(
                    out_ap[h, P * 512:(P + 1) * 512, :].rearrange(
                        "(c p) d -> p c d", p=128),
                    ostage[:].rearrange("p (c x) -> p c x", x=64))


_NC_CACHE = None


def _get_nc():
    global _NC_CACHE
    if _NC_CACHE is None:
        _NC_CACHE = _build()
    return _NC_CACHE


def kernel(q, k, v, mask, taus, attn_bias):
    q = np.asarray(q)
    k = np.asarray(k)
    v = np.asarray(v)
    mask = np.asarray(mask)
    taus = np.asarray(taus)
    attn_bias = np.asarray(attn_bias)

    nc = _get_nc()
    in_maps = []
    for c in range(N_CORES):
        b = c // 4
        h0 = (c % 4) * 4
        in_maps.append({
            "q": np.ascontiguousarray(q[b, h0:h0 + HPC], dtype=np.float32),
            "k": np.ascontiguousarray(k[b, h0:h0 + HPC], dtype=np.float32),
            "v": np.ascontiguousarray(v[b, h0:h0 + HPC], dtype=np.float32),
            "taus": np.ascontiguousarray(taus[h0:h0 + HPC], dtype=np.float32),
            "mask": np.ascontiguousarray(mask[b]).view(np.uint8),
            "bias": np.ascontiguousarray(attn_bias[b], dtype=np.float32),
        })

    res = bass_utils.run_bass_kernel_spmd(nc, in_maps, core_ids=list(range(N_CORES)))
    out = np.empty((B, H, S, D), dtype=np.float32)
    for c in range(N_CORES):
        b = c // 4
        h0 = (c % 4) * 4
        out[b, h0:h0 + HPC] = res.results[c]["out"]
    return out


if __name__ == "__main__":
    rng = np.random.default_rng(0)
    inputs = {
        "q": rng.standard_normal((B, H, S, D), dtype=np.float32),
        "k": rng.standard_normal((B, H, S, D), dtype=np.float32),
        "v": rng.standard_normal((B, H, S, D), dtype=np.float32),
        "mask": rng.random((B, S)) < 0.5,
        "taus": rng.random(H, dtype=np.float32),
        "attn_bias": rng.random((B, S, S), dtype=np.float32),
    }
    o = kernel(**inputs)
    print("out", o.shape, o.dtype, np.isfinite(o).all())


# revision 11
# speedup vs baseline: 36.7305x; 36.7305x over previous
"""Biased attention Trainium2 kernel, SPMD over 8 NeuronCores.

Problem (per reference):
    sim  = q @ k^T / sqrt(64)                       [b,h,i,j]
    sim  = where(mask[b,j], sim, -fmax)
    sim -= taus[h] * attn_bias[b,i,j]
    out  = softmax(sim, axis=j) @ v                 [b,h,i,d]

Shapes: B=2, H=16, S=2048, D=64, fp32.

Sharding: batch*heads across 8 cores -> 4 (b,h) pairs per core, all with
the same batch b (core c handles b=c//4, heads 4*(c%4)..4*(c%4)+3), so
attn_bias/mask are batch-sharded and loaded once per core.

Per-core dataflow (all on device):
  - scores are computed TRANSPOSED: zT[j,i] = K Q^T, j on partitions, so
    softmax reductions run along the matmul contraction instead of needing
    a big transpose of the attention matrix.
  - key-padding mask folds into the softmax exp as a per-partition bias
    (maskadd[j] = (mask-1)*1e30) on the ACT activation instruction.
  - the tau*attn_bias subtraction is done two ways, load-balanced between
    engines: PE path (scaled-identity matmul accumulating -8*tau*biasT
    into the scores PSUM) and DVE path (scalar_tensor_tensor fused
    (biasT * -8tau) + scores). 1/sqrt(d)=1/8 folds into the exp scale.
  - V gets a ones-column appended so the softmax denominator falls out of
    the attention @ V matmul for free (row 64 of the [65,512] output).
  - output is un-transposed per 128-column block with PE transpose into
    the freed accumulator bank, then normalized with a per-partition
    reciprocal multiply and DMA'd out.
"""

import numpy as np
from contextlib import ExitStack

import concourse.bass as bass
import concourse.tile as tile
from concourse import bacc, mybir
from concourse import bass_utils

F32 = mybir.dt.float32
F32R = mybir.dt.float32r
BF16 = mybir.dt.bfloat16
U8 = mybir.dt.uint8
Alu = mybir.AluOpType
Act = mybir.ActivationFunctionType

B, H, S, D = 2, 16, 2048, 64
N_CORES = 8
HPC = 4          # heads per core
JT = S // 128    # 16 j-tiles of 128
NP = S // 512    # 4 i-panels of 512
BIG = 1.0e30

# dtype used for matmul operands (PE fast fp32 path); falls back to F32 if
# hardware/toolchain rejects it -- see _build().
MM_DT = F32R
# exp output dtype (feeds the attention@V matmul as the moving operand)
EXP_DT = F32R
# which heads use the DVE scalar_tensor_tensor bias path per (panel, jtile):
# STT on pair0 every j balances PE/DVE/ACT at roughly 2.1us per (P,j) group.
def stt_pairs(j):
    return {0}          # pair 0 -> DVE path; pair 1 -> PE identity path


def _build(n_rep=1):
    nc = bacc.Bacc("TRN2", target_bir_lowering=False, debug=False,
                   num_devices=N_CORES)

    q_ap = nc.dram_tensor("q", [HPC, S, D], F32, kind="ExternalInput").ap()
    k_ap = nc.dram_tensor("k", [HPC, S, D], F32, kind="ExternalInput").ap()
    v_ap = nc.dram_tensor("v", [HPC, S, D], F32, kind="ExternalInput").ap()
    taus_ap = nc.dram_tensor("taus", [HPC], F32, kind="ExternalInput").ap()
    mask_ap = nc.dram_tensor("mask", [S], U8, kind="ExternalInput").ap()
    bias_ap = nc.dram_tensor("bias", [S, S], F32, kind="ExternalInput").ap()
    out_ap = nc.dram_tensor("out", [HPC, S, D], F32, kind="ExternalOutput").ap()

    with tile.TileContext(nc) as tc:
        for _rep in range(n_rep):
            with ExitStack() as ctx:
                _body(ctx, tc, q_ap, k_ap, v_ap, taus_ap, mask_ap, bias_ap,
                      out_ap)

    nc.compile()
    return nc


def _body(ctx, tc, q_ap, k_ap, v_ap, taus_ap, mask_ap, bias_ap, out_ap):
    nc = tc.nc

    const = ctx.enter_context(tc.tile_pool(name="const", bufs=1))
    tmp = ctx.enter_context(tc.tile_pool(name="tmp", bufs=2))
    braw = ctx.enter_context(tc.tile_pool(name="braw", bufs=18))
    benc = ctx.enter_context(tc.tile_pool(name="benc", bufs=3))
    zsb = ctx.enter_context(tc.tile_pool(name="zsb", bufs=3))
    epool = ctx.enter_context(tc.tile_pool(name="epool", bufs=3))
    dpool = ctx.enter_context(tc.tile_pool(name="dpool", bufs=2))
    zps = ctx.enter_context(tc.tile_pool(name="zps", bufs=2, space="PSUM"))
    ops = ctx.enter_context(tc.tile_pool(name="ops", bufs=1, space="PSUM"))

    # ---- constants / per-head prep -------------------------------------
    # taus broadcast to all partitions, scaled by -8: one K=1 matmul.
    taus_s = const.tile([1, HPC], F32, tag="taus_s")
    nc.sync.dma_start(taus_s[:], taus_ap[None, :])
    ones_n8 = const.tile([1, 128], F32, tag="ones_n8")
    nc.vector.memset(ones_n8[:], -8.0)
    zp0 = zps.tile([128, 1024], F32, tag="zp")
    nc.tensor.matmul(zp0[:, 0:HPC], lhsT=ones_n8[:], rhs=taus_s[:],
                     start=True, stop=True)
    n8tau = const.tile([128, HPC], F32, tag="n8tau")
    nc.vector.tensor_copy(n8tau[:], zp0[:, 0:HPC])

    # identity (for PE transpose) and per-head scaled identities (-8*tau*I)
    ones_t = const.tile([128, 128], F32, tag="ones_t")
    nc.vector.memset(ones_t[:], 1.0)
    ident = const.tile([128, 128], F32, tag="ident")
    nc.gpsimd.affine_select(ident[:], ones_t[:], pattern=[[1, 128]], base=0,
                            channel_multiplier=-1, compare_op=Alu.is_equal,
                            fill=0.0)
    scaledI = const.tile([128, 128 * HPC], MM_DT, tag="scaledI")
    for h in range(HPC):
        nc.vector.tensor_scalar_mul(scaledI[:, h * 128:(h + 1) * 128],
                                    ident[:], n8tau[:, h:h + 1])

    # maskadd[j] = (mask - 1) * BIG as [128, JT] fp32
    m_u8 = const.tile([128, JT], U8, tag="m_u8")
    nc.sync.dma_start(m_u8[:], mask_ap.rearrange("(t p) -> p t", p=128))
    m_f = const.tile([128, JT], F32, tag="m_f")
    nc.vector.tensor_copy(m_f[:], m_u8[:])
    maskadd = const.tile([128, JT], F32, tag="maskadd")
    nc.vector.tensor_scalar(maskadd[:], m_f[:], 1.0, BIG,
                            op0=Alu.subtract, op1=Alu.mult)

    # Q^T / K^T head-pair tiles [128, S]: even head on partitions 0-63,
    # odd head on 64-127 (enables row-packed concurrent QK matmuls).
    qtr = []
    ktr = []
    for pair in range(2):
        qraw = tmp.tile([128, S], F32, tag="qkraw")
        nc.sync.dma_start(qraw[0:64, :], q_ap[2 * pair].rearrange("s d -> d s"))
        nc.sync.dma_start(qraw[64:128, :], q_ap[2 * pair + 1].rearrange("s d -> d s"))
        qt = const.tile([128, S], MM_DT, tag=f"qtr{pair}")
        nc.vector.tensor_copy(qt[:], qraw[:])
        qtr.append(qt)

        kraw = tmp.tile([128, S], F32, tag="qkraw")
        nc.sync.dma_start(kraw[0:64, :], k_ap[2 * pair].rearrange("s d -> d s"))
        nc.sync.dma_start(kraw[64:128, :], k_ap[2 * pair + 1].rearrange("s d -> d s"))
        kt = const.tile([128, S], MM_DT, tag=f"ktr{pair}")
        nc.vector.tensor_copy(kt[:], kraw[:])
        ktr.append(kt)

    # V with ones column: [128, JT*65] per head, col t*65+64 == 1.0
    vaug = []
    for h in range(HPC):
        vraw = tmp.tile([128, JT * 65], F32, tag="vraw")
        nc.vector.memset(vraw[:], 1.0)
        dst = vraw[:].rearrange("p (t c) -> p t c", c=65)[:, :, 0:64]
        nc.sync.dma_start(dst, v_ap[h].rearrange("(t p) d -> p t d", p=128))
        va = const.tile([128, JT * 65], MM_DT, tag=f"vaug{h}")
        nc.vector.tensor_copy(va[:], vraw[:])
        vaug.append(va)

    # ---- main loops ----------------------------------------------------
    # Head pairs are processed sequentially per panel (phase 0: heads 0/1,
    # phase 1: heads 2/3) so only 2 PSUM o-banks are live per phase and the
    # scores pool gets 2 [128,1024] slots for pipelining. Bias tiles are
    # cached raw in SBUF per panel and shared by both phases; the tau*bias
    # application alternates DVE (scalar_tensor_tensor) / PE (scaled
    # identity matmul) by j-parity to balance engine load.
    for P in range(NP):
        isl = slice(P * 512, (P + 1) * 512)
        bcache = [None] * JT
        for j in range(JT):
            jsl = slice(j * 128, (j + 1) * 128)
            bT = braw.tile([128, 512], F32, tag="bT", name=f"bT_{P}_{j}")
            nc.sync.dma_start(bT[:], bias_ap.rearrange("i j -> j i")[jsl, isl])
            bcache[j] = bT

        for pair in range(2):
            o = [ops.tile([128, 512], F32, tag=f"o{h}", name=f"o{h}_{P}")
                 for h in (2 * pair, 2 * pair + 1)]
            for j in range(JT):
                jsl = slice(j * 128, (j + 1) * 128)
                bT = bcache[j]
                use_stt = (j % 2 == 0)
                zp = zps.tile([128, 1024], F32, tag="zp", name=f"zp_{P}_{pair}_{j}")
                if not use_stt:
                    bTr = benc.tile([128, 512], MM_DT, tag="bTr", name=f"bTr_{P}_{pair}_{j}")
                    nc.vector.tensor_copy(bTr[:], bT[:])
                for t in range(2):
                    h = 2 * pair + t
                    psl = slice(t * 64, (t + 1) * 64)
                    zsl = slice(t * 512, (t + 1) * 512)
                    nc.tensor.matmul(zp[:, zsl], lhsT=ktr[pair][psl, jsl],
                                     rhs=qtr[pair][psl, isl],
                                     start=True, stop=use_stt)
                    if not use_stt:
                        nc.tensor.matmul(
                            zp[:, zsl],
                            lhsT=scaledI[:, h * 128:(h + 1) * 128],
                            rhs=bTr[:], start=False, stop=True,
                            skip_group_check=True)
                et = epool.tile([128, 1024], EXP_DT, tag="et", name=f"et_{P}_{pair}_{j}")
                if use_stt:
                    zs = zsb.tile([128, 1024], F32, tag="zs", name=f"zs_{P}_{pair}_{j}")
                    for t in range(2):
                        h = 2 * pair + t
                        zsl = slice(t * 512, (t + 1) * 512)
                        nc.vector.scalar_tensor_tensor(
                            zs[:, zsl], in0=bT[:], scalar=n8tau[:, h:h + 1],
                            in1=zp[:, zsl], op0=Alu.mult, op1=Alu.add)
                    nc.scalar.activation(et[:], zs[:], Act.Exp,
                                         bias=maskadd[:, j:j + 1], scale=0.125)
                else:
                    nc.scalar.activation(et[:], zp[:], Act.Exp,
                                         bias=maskadd[:, j:j + 1], scale=0.125)
                for t in range(2):
                    h = 2 * pair + t
                    nc.tensor.matmul(
                        o[t][0:65, :],
                        lhsT=vaug[h][:, j * 65:(j + 1) * 65],
                        rhs=et[:, t * 512:(t + 1) * 512],
                        start=(j == 0), stop=(j == JT - 1))

            # ---- drain: transpose + normalize + store ------------------
            for t in range(2):
                h = 2 * pair + t
                ob = dpool.tile([65, 512], F32, tag="ob", name=f"ob_{P}_{h}")
                nc.vector.tensor_copy(ob[:], o[t][0:65, :])
                for c in range(4):
                    nc.tensor.transpose(o[t][:, c * 65:(c + 1) * 65],
                                        ob[:, c * 128:(c + 1) * 128],
                                        ident[0:65, 0:65])
                oc = o[t][:, 0:260].rearrange("p (c x) -> p c x", x=65)
                rec = dpool.tile([128, 4], F32, tag="rec", name=f"rec_{P}_{h}")
                nc.vector.reciprocal(rec[:], oc[:, :, 64])
                ostage = dpool.tile([128, 256], F32, tag="ostage",
                                    name=f"ostage_{P}_{h}")
                nc.vector.tensor_tensor(
                    ostage[:].rearrange("p (c x) -> p c x", x=64),
                    oc[:, :, 0:64],
                    rec[:].broadcast_to((128, 4, 64)),
                    op=Alu.mult)
                nc.sync.dma_start(
                    out_ap[h, P * 512:(P + 1) * 512, :].rearrange(
                        "(c p) d -> p c d", p=128),
                    ostage[:].rearrange("p (c x) -> p c x", x=64))


_NC_CACHE = None


def _get_nc():
    global _NC_CACHE
    if _NC_CACHE is None:
        _NC_CACHE = _build()
    return _NC_CACHE


def kernel(q, k, v, mask, taus, attn_bias):
    q = np.asarray(q)
    k = np.asarray(k)
    v = np.asarray(v)
    mask = np.asarray(mask)
    taus = np.asarray(taus)
    attn_bias = np.asarray(attn_bias)

    nc = _get_nc()
    in_maps = []
    for c in range(N_CORES):
        b = c // 4
        h0 = (c % 4) * 4
        in_maps.append({
            "q": np.ascontiguousarray(q[b, h0:h0 + HPC], dtype=np.float32),
            "k": np.ascontiguousarray(k[b, h0:h0 + HPC], dtype=np.float32),
            "v": np.ascontiguousarray(v[b, h0:h0 + HPC], dtype=np.float32),
            "taus": np.ascontiguousarray(taus[h0:h0 + HPC], dtype=np.float32),
            "mask": np.ascontiguousarray(mask[b]).view(np.uint8),
            "bias": np.ascontiguousarray(attn_bias[b], dtype=np.float32),
        })

    res = bass_utils.run_bass_kernel_spmd(nc, in_maps, core_ids=list(range(N_CORES)))
    out = np.empty((B, H, S, D), dtype=np.float32)
    for c in range(N_CORES):
        b = c // 4
        h0 = (c % 4) * 4
        out[b, h0:h0 + HPC] = res.results[c]["out"]
    return out


if __name__ == "__main__":
    rng = np.random.default_rng(0)
    inputs = {
        "q": rng.standard_normal((B, H, S, D), dtype=np.float32),
        "k": rng.standard_normal((B, H, S, D), dtype=np.float32),
        "v": rng.standard_normal((B, H, S, D), dtype=np.float32),
        "mask": rng.random((B, S)) < 0.5,
        "taus": rng.random(H, dtype=np.float32),
        "attn_bias": rng.random((B, S, S), dtype=np.float32),
    }
    o = kernel(**inputs)
    print("out", o.shape, o.dtype, np.isfinite(o).all())


# revision 15
# speedup vs baseline: 498.7149x; 13.5777x over previous
"""Biased attention Trainium2 kernel, SPMD over 8 NeuronCores.

Problem (per reference):
    sim  = q @ k^T / sqrt(64)                       [b,h,i,j]
    sim  = where(mask[b,j], sim, -fmax)
    sim -= taus[h] * attn_bias[b,i,j]
    out  = softmax(sim, axis=j) @ v                 [b,h,i,d]

Shapes: B=2, H=16, S=2048, D=64, fp32.

Sharding: batch*heads across 8 cores -> 4 (b,h) pairs per core, all with
the same batch b (core c handles b=c//4, heads 4*(c%4)..4*(c%4)+3), so
attn_bias/mask are batch-sharded and loaded once per core.

Per-core dataflow (all on device):
  - scores are computed TRANSPOSED: zT[j,i] = K Q^T, j on partitions, so
    softmax reductions run along the matmul contraction instead of needing
    a big transpose of the attention matrix.
  - key-padding mask folds into the softmax exp as a per-partition bias
    (maskadd[j] = (mask-1)*1e30) on the ACT activation instruction.
  - the tau*attn_bias subtraction is done two ways, load-balanced between
    engines: PE path (scaled-identity matmul accumulating -8*tau*biasT
    into the scores PSUM) and DVE path (scalar_tensor_tensor fused
    (biasT * -8tau) + scores). 1/sqrt(d)=1/8 folds into the exp scale.
  - V gets a ones-column appended so the softmax denominator falls out of
    the attention @ V matmul for free (row 64 of the [65,512] output).
  - output is un-transposed per 128-column block with PE transpose into
    the freed accumulator bank, then normalized with a per-partition
    reciprocal multiply and DMA'd out.
"""

import numpy as np
from contextlib import ExitStack

import concourse.bass as bass
import concourse.tile as tile
from concourse import bacc, mybir
from concourse import bass_utils

F32 = mybir.dt.float32
F32R = mybir.dt.float32r
BF16 = mybir.dt.bfloat16
U8 = mybir.dt.uint8
Alu = mybir.AluOpType
Act = mybir.ActivationFunctionType

B, H, S, D = 2, 16, 2048, 64
N_CORES = 8
HPC = 4          # heads per core
JT = S // 128    # 16 j-tiles of 128
NP = S // 512    # 4 i-panels of 512
BIG = 1.0e30

# dtype used for matmul operands (PE fast fp32 path); falls back to F32 if
# hardware/toolchain rejects it -- see _build().
MM_DT = F32R
# exp output dtype (feeds the attention@V matmul as the moving operand)
EXP_DT = F32R
def _build(n_rep=1):
    nc = bacc.Bacc("TRN2", target_bir_lowering=False, debug=False,
                   num_devices=N_CORES)

    q_ap = nc.dram_tensor("qt", [HPC, D, S], F32, kind="ExternalInput").ap()
    k_ap = nc.dram_tensor("kt", [HPC, D, S], F32, kind="ExternalInput").ap()
    v_ap = nc.dram_tensor("vp", [HPC, 128, (S // 128) * 65], F32,
                          kind="ExternalInput").ap()
    taus_ap = nc.dram_tensor("taus", [HPC], F32, kind="ExternalInput").ap()
    mask_ap = nc.dram_tensor("maskT", [128, S // 128], U8, kind="ExternalInput").ap()
    bias_ap = nc.dram_tensor("biasT", [S, S], F32, kind="ExternalInput").ap()
    out_ap = nc.dram_tensor("out", [HPC, S // 512, 128, 256], F32,
                            kind="ExternalOutput").ap()

    with tile.TileContext(nc) as tc:
        for _rep in range(n_rep):
            with ExitStack() as ctx:
                _body(ctx, tc, q_ap, k_ap, v_ap, taus_ap, mask_ap, bias_ap,
                      out_ap)

    nc.compile()
    return nc


def _body(ctx, tc, q_ap, k_ap, v_ap, taus_ap, mask_ap, bias_ap, out_ap):
    nc = tc.nc

    const = ctx.enter_context(tc.tile_pool(name="const", bufs=1))
    tmp = ctx.enter_context(tc.tile_pool(name="tmp", bufs=2))
    braw = ctx.enter_context(tc.tile_pool(name="braw", bufs=18))
    benc = ctx.enter_context(tc.tile_pool(name="benc", bufs=3))
    zsb = ctx.enter_context(tc.tile_pool(name="zsb", bufs=3))
    epool = ctx.enter_context(tc.tile_pool(name="epool", bufs=3))
    dpool = ctx.enter_context(tc.tile_pool(name="dpool", bufs=2))
    zps = ctx.enter_context(tc.tile_pool(name="zps", bufs=3, space="PSUM"))
    ops = ctx.enter_context(tc.tile_pool(name="ops", bufs=1, space="PSUM"))

    # ---- constants / per-head prep -------------------------------------
    # taus broadcast to all partitions, scaled by -8: one K=1 matmul.
    taus_s = const.tile([1, HPC], F32, tag="taus_s")
    nc.sync.dma_start(taus_s[:], taus_ap[None, :])
    ones_n8 = const.tile([1, 128], F32, tag="ones_n8")
    nc.vector.memset(ones_n8[:], -8.0)
    zp0 = zps.tile([128, 1024], F32, tag="zp")
    nc.tensor.matmul(zp0[:, 0:HPC], lhsT=ones_n8[:], rhs=taus_s[:],
                     start=True, stop=True)
    n8tau = const.tile([128, HPC], F32, tag="n8tau")
    nc.vector.tensor_copy(n8tau[:], zp0[:, 0:HPC])

    # identity (for PE transpose) and per-head scaled identities (-8*tau*I)
    ones_t = const.tile([128, 128], F32, tag="ones_t")
    nc.vector.memset(ones_t[:], 1.0)
    ident = const.tile([128, 128], F32, tag="ident")
    nc.gpsimd.affine_select(ident[:], ones_t[:], pattern=[[1, 128]], base=0,
                            channel_multiplier=-1, compare_op=Alu.is_equal,
                            fill=0.0)
    scaledI = const.tile([128, 128 * HPC], MM_DT, tag="scaledI")
    for h in range(HPC):
        nc.vector.tensor_scalar_mul(scaledI[:, h * 128:(h + 1) * 128],
                                    ident[:], n8tau[:, h:h + 1])

    # maskadd[j] = (mask - 1) * BIG as [128, JT] fp32
    m_u8 = const.tile([128, JT], U8, tag="m_u8")
    nc.sync.dma_start(m_u8[:], mask_ap[:, :])
    m_f = const.tile([128, JT], F32, tag="m_f")
    nc.vector.tensor_copy(m_f[:], m_u8[:])
    maskadd = const.tile([128, JT], F32, tag="maskadd")
    nc.vector.tensor_scalar(maskadd[:], m_f[:], 1.0, BIG,
                            op0=Alu.subtract, op1=Alu.mult)

    # Q^T / K^T head-pair tiles [128, S]: even head on partitions 0-63,
    # odd head on 64-127 (enables row-packed concurrent QK matmuls).
    qtr = []
    ktr = []
    for pair in range(2):
        qraw = tmp.tile([128, S], F32, tag="qkraw")
        nc.sync.dma_start(qraw[0:64, :], q_ap[2 * pair])
        nc.sync.dma_start(qraw[64:128, :], q_ap[2 * pair + 1])
        qt = const.tile([128, S], MM_DT, tag=f"qtr{pair}")
        nc.vector.tensor_copy(qt[:], qraw[:])
        qtr.append(qt)

        kraw = tmp.tile([128, S], F32, tag="qkraw")
        nc.sync.dma_start(kraw[0:64, :], k_ap[2 * pair])
        nc.sync.dma_start(kraw[64:128, :], k_ap[2 * pair + 1])
        kt = const.tile([128, S], MM_DT, tag=f"ktr{pair}")
        nc.vector.tensor_copy(kt[:], kraw[:])
        ktr.append(kt)

    # V with ones column, host-premarshalled [128, JT*65] per head
    vaug = []
    for h in range(HPC):
        vraw = tmp.tile([128, JT * 65], F32, tag="vraw")
        nc.sync.dma_start(vraw[:], v_ap[h])
        va = const.tile([128, JT * 65], MM_DT, tag=f"vaug{h}")
        nc.vector.tensor_copy(va[:], vraw[:])
        vaug.append(va)

    # ---- main loops ----------------------------------------------------
    # Head pairs are processed sequentially per panel (phase 0: heads 0/1,
    # phase 1: heads 2/3) so only 2 PSUM o-banks are live per phase and the
    # scores pool gets 2 [128,1024] slots for pipelining. Bias tiles are
    # cached raw in SBUF per panel and shared by both phases; the tau*bias
    # application alternates DVE (scalar_tensor_tensor) / PE (scaled
    # identity matmul) by j-parity to balance engine load.
    for P in range(NP):
        isl = slice(P * 512, (P + 1) * 512)
        bcache = [None] * JT
        for j in range(JT):
            jsl = slice(j * 128, (j + 1) * 128)
            bT = braw.tile([128, 512], F32, tag="bT", name=f"bT_{P}_{j}")
            nc.sync.dma_start(bT[:], bias_ap[jsl, isl])
            bcache[j] = bT

        for pair in range(2):
            o = [ops.tile([128, 512], F32, tag=f"o{t}", name=f"o{t}_{P}_{pair}")
                 for t in range(2)]
            for j in range(JT):
                jsl = slice(j * 128, (j + 1) * 128)
                bT = bcache[j]
                use_stt = (j % 2 == 0)
                zp = zps.tile([128, 1024], F32, tag="zp", name=f"zp_{P}_{pair}_{j}")
                if not use_stt:
                    bTr = benc.tile([128, 512], MM_DT, tag="bTr", name=f"bTr_{P}_{pair}_{j}")
                    nc.vector.tensor_copy(bTr[:], bT[:])
                for t in range(2):
                    h = 2 * pair + t
                    psl = slice(t * 64, (t + 1) * 64)
                    zsl = slice(t * 512, (t + 1) * 512)
                    nc.tensor.matmul(zp[:, zsl], lhsT=ktr[pair][psl, jsl],
                                     rhs=qtr[pair][psl, isl],
                                     start=True, stop=use_stt)
                    if not use_stt:
                        nc.tensor.matmul(
                            zp[:, zsl],
                            lhsT=scaledI[:, h * 128:(h + 1) * 128],
                            rhs=bTr[:], start=False, stop=True,
                            skip_group_check=True)
                et = epool.tile([128, 1024], EXP_DT, tag="et", name=f"et_{P}_{pair}_{j}")
                if use_stt:
                    zs = zsb.tile([128, 1024], F32, tag="zs", name=f"zs_{P}_{pair}_{j}")
                    for t in range(2):
                        h = 2 * pair + t
                        zsl = slice(t * 512, (t + 1) * 512)
                        nc.vector.scalar_tensor_tensor(
                            zs[:, zsl], in0=bT[:], scalar=n8tau[:, h:h + 1],
                            in1=zp[:, zsl], op0=Alu.mult, op1=Alu.add)
                    nc.scalar.activation(et[:], zs[:], Act.Exp,
                                         bias=maskadd[:, j:j + 1], scale=0.125)
                else:
                    nc.scalar.activation(et[:], zp[:], Act.Exp,
                                         bias=maskadd[:, j:j + 1], scale=0.125)
                for t in range(2):
                    h = 2 * pair + t
                    nc.tensor.matmul(
                        o[t][0:65, :],
                        lhsT=vaug[h][:, j * 65:(j + 1) * 65],
                        rhs=et[:, t * 512:(t + 1) * 512],
                        start=(j == 0), stop=(j == JT - 1))

            # ---- drain: transpose + normalize + store ------------------
            for t in range(2):
                h = 2 * pair + t
                ob = dpool.tile([65, 512], F32, tag="ob", name=f"ob_{P}_{h}")
                nc.vector.tensor_copy(ob[:], o[t][0:65, :])
                for c in range(4):
                    nc.tensor.transpose(o[t][:, c * 65:(c + 1) * 65],
                                        ob[:, c * 128:(c + 1) * 128],
                                        ident[0:65, 0:65])
                oc = o[t][:, 0:260].rearrange("p (c x) -> p c x", x=65)
                rec = dpool.tile([128, 4], F32, tag="rec", name=f"rec_{P}_{h}")
                nc.vector.reciprocal(rec[:], oc[:, :, 64])
                ostage = dpool.tile([128, 256], F32, tag="ostage",
                                    name=f"ostage_{P}_{h}")
                nc.vector.tensor_tensor(
                    ostage[:].rearrange("p (c x) -> p c x", x=64),
                    oc[:, :, 0:64],
                    rec[:].broadcast_to((128, 4, 64)),
                    op=Alu.mult)
                nc.sync.dma_start(out_ap[h, P], ostage[:])


_NC_CACHE = None


def _get_nc():
    global _NC_CACHE
    if _NC_CACHE is None:
        _NC_CACHE = _build()
    return _NC_CACHE


def vprep(vh):
    # [HPC, S, D] -> [HPC, 128, JT*65] : partition p holds rows {128*t+p},
    # 65th column per j-tile is 1.0 (softmax denominator ones-row)
    vh = np.asarray(vh, dtype=np.float32).reshape(HPC, JT, 128, D)
    out = np.ones((HPC, 128, JT, 65), dtype=np.float32)
    out[:, :, :, 0:64] = vh.transpose(0, 2, 1, 3)
    return np.ascontiguousarray(out.reshape(HPC, 128, JT * 65))


def kernel(q, k, v, mask, taus, attn_bias):
    q = np.asarray(q)
    k = np.asarray(k)
    v = np.asarray(v)
    mask = np.asarray(mask)
    taus = np.asarray(taus)
    attn_bias = np.asarray(attn_bias)

    nc = _get_nc()
    biasT = [np.ascontiguousarray(attn_bias[b].T, dtype=np.float32)
             for b in range(B)]
    in_maps = []
    for c in range(N_CORES):
        b = c // 4
        h0 = (c % 4) * 4
        in_maps.append({
            "qt": np.ascontiguousarray(
                q[b, h0:h0 + HPC].swapaxes(1, 2), dtype=np.float32),
            "kt": np.ascontiguousarray(
                k[b, h0:h0 + HPC].swapaxes(1, 2), dtype=np.float32),
            "vp": vprep(v[b, h0:h0 + HPC]),
            "taus": np.ascontiguousarray(taus[h0:h0 + HPC], dtype=np.float32),
            "maskT": np.ascontiguousarray(
                mask[b].reshape(S // 128, 128).T).view(np.uint8),
            "biasT": biasT[b],
        })

    res = bass_utils.run_bass_kernel_spmd(nc, in_maps, core_ids=list(range(N_CORES)))
    out = np.empty((B, H, S, D), dtype=np.float32)
    for c in range(N_CORES):
        b = c // 4
        h0 = (c % 4) * 4
        arr = res.results[c]["out"].reshape(HPC, NP, 128, 4, 64)
        # i = P*512 + c*128 + p  ->  [h, (P, c, p), d]
        out[b, h0:h0 + HPC] = arr.transpose(0, 1, 3, 2, 4).reshape(HPC, S, D)
    return out


if __name__ == "__main__":
    rng = np.random.default_rng(0)
    inputs = {
        "q": rng.standard_normal((B, H, S, D), dtype=np.float32),
        "k": rng.standard_normal((B, H, S, D), dtype=np.float32),
        "v": rng.standard_normal((B, H, S, D), dtype=np.float32),
        "mask": rng.random((B, S)) < 0.5,
        "taus": rng.random(H, dtype=np.float32),
        "attn_bias": rng.random((B, S, S), dtype=np.float32),
    }
    o = kernel(**inputs)
    print("out", o.shape, o.dtype, np.isfinite(o).all())
